# revision 10
# baseline (speedup 1.0000x reference)
"""Trainium2 Bass kernel for nn_MultiHeadAttention_5360119185803.

Full-d_model attention (no head split) + residual + LayerNorm, B=4, T=S=2048,
E=1024, fp32 in/out.

Sharding: 8 cores; core c owns batch b=c//2 and query rows
[(c%2)*1024, (c%2+1)*1024). Each core needs the full key/value of its batch
(K/V projection duplicated across the core pair) — no collectives.

Per-core device pipeline (all matmuls in float32r = TF32-like, full PE rate):
  P1  kT = (Wk.T).T @ xk.T           [f,s]
      (activations are transposed via 2-byte xbar DMA-transpose of a host-
      side bf16 hi/lo split, recombined hi+lo on DVE into f32r — keeps
      ~16-17 mantissa bits and zero PE time; PE transposes were 90us)
  P2  v  = xv @ Wv.T  -> spilled to DRAM (SBUF pressure), bias bv folded into
      bo' = bo + Wo@bv on host (attn rows sum to 1)
  P3  qT = (Wq.T/32).T @ xq.T + bq/32  [f,t]  (1/sqrt(E) folded into Wq, bq)
  P4  scoresT[s,t] = kT.T @ qT  (PSUM) -> expT = exp(scoresT)  (ACT, no
      max-subtraction: |scores/32| <~ 6 so exp is safe in fp32; bk dropped
      entirely — it shifts scores by a per-t constant, softmax-invariant)
      rowsum[1,t] = ones.T @ expT (PE), redistributed to [128,8] per-partition
  P5  ctxT[e',t] = sum_s v[s,e'] * expT[s,t]  (8 PSUM banks per t-half)
  P6  out[t,g] = (ctxT.T @ Wo.T) * (1/rowsum)[t] + bo'+ residual; LayerNorm
      over g on DVE (bn_stats/bn_aggr); gamma/beta applied only if non-trivial.

kernel() is self-contained: host prep = shard + weight transposes/scale folds.
"""

import sys

sys.path.insert(0, "/opt/trn_rl_repo")

import numpy as np

import ml_dtypes

import concourse.bacc as bacc
import concourse.bass as bass
import concourse.tile as tile
from concourse import mybir
from concourse.bass_utils import run_bass_kernel_spmd

BF16 = ml_dtypes.bfloat16

P = 128
E = 1024          # d_model
S = 2048          # kv seq len per batch
T = 1024          # query rows per core
NE = E // P       # 8 chunks of contraction dim
NT = T // P       # 8 t tiles
NS = S // P       # 16 s tiles
FD = 512          # matmul moving free dim / PSUM bank
NBLK_S = S // FD  # 4 s blocks
NBLK_T = T // FD  # 2 t blocks

f32 = mybir.dt.float32
f32r = mybir.dt.float32r
bf16 = mybir.dt.bfloat16
AF = mybir.ActivationFunctionType
ALU = mybir.AluOpType

_cache = {}


def _load_weight(nc, pool, dram):
    """[E, x] fp32 DRAM -> [128, NE, x] f32r SBUF (gpsimd DMA casts+rounds)."""
    w = pool.tile([P, NE, E], f32r)
    v = dram.ap().rearrange("(j p) f -> j p f", p=P)
    for j in range(NE):
        nc.gpsimd.dma_start(out=w[:, j, :], in_=v[j])
    return w


def _load_xt_block(nc, xt_blk, x_hi, x_lo, row0, nrows, hilo_pool):
    """xt_blk[:, j, :] (f32r [128, NE, nrows]) = x[row0:row0+nrows,
    j*128:(j+1)*128].T via two 2-byte xbar DMA-transposes + DVE hi+lo add.
    nc.sync is reserved for transposed DMAs (xbar-mode transitions serialize,
    so plain DMAs ride nc.scalar/gpsimd instead)."""
    for j in range(NE):
        th = hilo_pool.tile([P, nrows], bf16, name=f"th{row0}_{j}", tag="th")
        nc.sync.dma_start(
            out=th, in_=x_hi.ap()[row0:row0 + nrows, j * P:(j + 1) * P],
            transpose=True)
        tl = hilo_pool.tile([P, nrows], bf16, name=f"tl{row0}_{j}", tag="tl")
        nc.sync.dma_start(
            out=tl, in_=x_lo.ap()[row0:row0 + nrows, j * P:(j + 1) * P],
            transpose=True)
        nc.vector.tensor_tensor(xt_blk[:, j, :], th, tl, ALU.add)


def _build(apply_gb):
    nc = bacc.Bacc("TRN2", target_bir_lowering=False, debug=False, num_devices=8)

    xq = nc.dram_tensor("xq", [T, E], f32, kind="ExternalInput")
    xq_hi = nc.dram_tensor("xq_hi", [T, E], bf16, kind="ExternalInput")
    xq_lo = nc.dram_tensor("xq_lo", [T, E], bf16, kind="ExternalInput")
    xk_hi = nc.dram_tensor("xk_hi", [S, E], bf16, kind="ExternalInput")
    xk_lo = nc.dram_tensor("xk_lo", [S, E], bf16, kind="ExternalInput")
    xv_hi = nc.dram_tensor("xv_hi", [S, E], bf16, kind="ExternalInput")
    xv_lo = nc.dram_tensor("xv_lo", [S, E], bf16, kind="ExternalInput")
    wqt = nc.dram_tensor("wqt", [E, E], f32, kind="ExternalInput")  # Wq.T/32 [e,f]
    wkt = nc.dram_tensor("wkt", [E, E], f32, kind="ExternalInput")  # Wk.T   [e,f]
    wvt = nc.dram_tensor("wvt", [E, E], f32, kind="ExternalInput")  # Wv.T   [e,e']
    wot = nc.dram_tensor("wot", [E, E], f32, kind="ExternalInput")  # Wo.T   [e',g]
    bq2 = nc.dram_tensor("bq2", [P, NE], f32, kind="ExternalInput")  # bq/32 tiled
    bo2 = nc.dram_tensor("bo2", [E], f32, kind="ExternalInput")      # bo + Wo@bv
    if apply_gb:
        gam = nc.dram_tensor("gam", [E], f32, kind="ExternalInput")
        bet = nc.dram_tensor("bet", [E], f32, kind="ExternalInput")
    out = nc.dram_tensor("out", [T, E], f32, kind="ExternalOutput")
    vsp = nc.dram_tensor("v_spill", [S, E], f32r)
    rs_dram = nc.dram_tensor("rs_scratch", [T], f32)

    with tile.TileContext(nc) as tc:
        consts = tc.alloc_tile_pool(name="consts", bufs=1, side="left")
        eps_t = consts.tile([P, 1], f32)
        nc.vector.memset(eps_t, 1e-6)
        ones_f = consts.tile([P, 1], f32)
        nc.vector.memset(ones_f, 1.0)
        ones_r = consts.tile([P, 1], f32r)
        nc.vector.tensor_copy(ones_r, ones_f)
        recip_t = consts.tile([P, NT], f32)

        kT_pool = tc.alloc_tile_pool(name="kT", bufs=1, side="left")
        kT = kT_pool.tile([P, NE, S], f32r)  # [f, fchunk, s] 8MB
        qT_pool = tc.alloc_tile_pool(name="qT", bufs=1, side="left")
        qT = qT_pool.tile([P, NE, T], f32r)  # [f, fchunk, t] 4MB
        identp = tc.alloc_tile_pool(name="identp", bufs=1, side="left")
        bq_sb = identp.tile([P, NE], f32)
        nc.scalar.dma_start(out=bq_sb, in_=bq2.ap())

        # ---- P1: kT projection ----
        with (
            tc.tile_pool(name="wk", bufs=1) as wkp,
            tc.tile_pool(name="p1hl", bufs=3) as hlp,
            tc.tile_pool(name="p1xt", bufs=2) as xtp,
            tc.tile_pool(name="p1mm", bufs=3, space="PSUM") as mmp,
        ):
            wk_sb = _load_weight(nc, wkp, wkt)
            for sb in range(NBLK_S):
                xt_blk = xtp.tile([P, NE, FD], f32r)
                _load_xt_block(nc, xt_blk, xk_hi, xk_lo, sb * FD, FD, hlp)
                for ft in range(NE):
                    ps = mmp.tile([P, FD], f32)
                    for j in range(NE):
                        nc.tensor.matmul(ps, wk_sb[:, j, ft * P:(ft + 1) * P],
                                         xt_blk[:, j, :],
                                         start=(j == 0), stop=(j == NE - 1))
                    nc.vector.tensor_copy(kT[:, ft, sb * FD:(sb + 1) * FD], ps)

        # ---- P2: v projection -> DRAM spill ----
        with (
            tc.tile_pool(name="wv", bufs=1) as wvp,
            tc.tile_pool(name="p2hl", bufs=3) as hlp,
            tc.tile_pool(name="p2xt", bufs=2) as xtp,
            tc.tile_pool(name="p2mm", bufs=3, space="PSUM") as mmp,
            tc.tile_pool(name="p2ev", bufs=3) as evp,
        ):
            wv_sb = _load_weight(nc, wvp, wvt)
            for sb in range(NBLK_S):
                xt_blk = xtp.tile([P, NE, FD], f32r)
                _load_xt_block(nc, xt_blk, xv_hi, xv_lo, sb * FD, FD, hlp)
                for ss in range(FD // P):
                    ev = evp.tile([P, E], f32r)
                    for ec in range(E // FD):
                        ps = mmp.tile([P, FD], f32)
                        for j in range(NE):
                            nc.tensor.matmul(ps, xt_blk[:, j, ss * P:(ss + 1) * P],
                                             wv_sb[:, j, ec * FD:(ec + 1) * FD],
                                             start=(j == 0), stop=(j == NE - 1))
                        nc.vector.tensor_copy(ev[:, ec * FD:(ec + 1) * FD], ps)
                    r0 = sb * FD + ss * P
                    nc.scalar.dma_start(out=vsp.ap()[r0:r0 + P, :], in_=ev)

        # ---- P3: qT projection (+bq/32) ----
        with (
            tc.tile_pool(name="wq", bufs=1) as wqp,
            tc.tile_pool(name="p3hl", bufs=3) as hlp,
            tc.tile_pool(name="p3xt", bufs=2) as xtp,
            tc.tile_pool(name="p3mm", bufs=3, space="PSUM") as mmp,
        ):
            wq_sb = _load_weight(nc, wqp, wqt)
            for tb in range(NBLK_T):
                xt_blk = xtp.tile([P, NE, FD], f32r)
                _load_xt_block(nc, xt_blk, xq_hi, xq_lo, tb * FD, FD, hlp)
                for ft in range(NE):
                    ps = mmp.tile([P, FD], f32)
                    for j in range(NE):
                        nc.tensor.matmul(ps, wq_sb[:, j, ft * P:(ft + 1) * P],
                                         xt_blk[:, j, :],
                                         start=(j == 0), stop=(j == NE - 1))
                    nc.vector.tensor_scalar(
                        out=qT[:, ft, tb * FD:(tb + 1) * FD], in0=ps,
                        scalar1=bq_sb[:, ft:ft + 1], scalar2=None, op0=ALU.add)

        identp.release()

        # ---- P4: scoresT -> expT; rowsum -> recip ----
        ctxT_pool = tc.alloc_tile_pool(name="ctxT", bufs=1, side="right")
        ctxT = ctxT_pool.tile([P, NE, T], f32r)  # [e', echunk, t] 4MB
        expT_pool = tc.alloc_tile_pool(name="expT", bufs=1, side="right")
        expT = expT_pool.tile([P, NS, T], f32r)  # [s, stile, t] 8MB
        with tc.tile_pool(name="p4mm", bufs=4, space="PSUM") as mmp:
            for st in range(NS):
                for tb in range(NBLK_T):
                    ps = mmp.tile([P, FD], f32)
                    for j in range(NE):
                        nc.tensor.matmul(ps, kT[:, j, st * P:(st + 1) * P],
                                         qT[:, j, tb * FD:(tb + 1) * FD],
                                         start=(j == 0), stop=(j == NE - 1))
                    nc.scalar.activation(expT[:, st, tb * FD:(tb + 1) * FD], ps, AF.Exp)

        with (
            tc.tile_pool(name="p4rs", bufs=2, space="PSUM") as rsp,
            tc.tile_pool(name="p4rw", bufs=1, side="right") as rwp,
        ):
            rs_sb = rwp.tile([1, T], f32)
            for tb in range(NBLK_T):
                ps = rsp.tile([P, FD], f32)
                for st in range(NS):
                    nc.tensor.matmul(ps[0:1, :], ones_r[:, 0:1],
                                     expT[:, st, tb * FD:(tb + 1) * FD],
                                     start=(st == 0), stop=(st == NS - 1))
                nc.vector.tensor_copy(rs_sb[0:1, tb * FD:(tb + 1) * FD], ps[0:1, :])
            nc.scalar.dma_start(out=rs_dram.ap(), in_=rs_sb[0:1, :])
            rsT = rwp.tile([P, NT], f32)
            nc.scalar.dma_start(out=rsT, in_=rs_dram.ap().rearrange("(j p) -> p j", p=P))
            nc.vector.reciprocal(recip_t, rsT)

        qT_pool.release()
        kT_pool.release()

        # ---- P5: ctxT ----
        with (
            tc.tile_pool(name="p5v", bufs=3, side="right") as vp,
            tc.tile_pool(name="p5mm", bufs=1, space="PSUM") as mmp,
        ):
            for tb in range(NBLK_T):
                pss = [mmp.tile([P, FD], f32, name=f"ctxps{tb}_{e}",
                                tag=f"ctxps{e}") for e in range(NE)]
                for st in range(NS):
                    vt = vp.tile([P, E], f32r)
                    nc.scalar.dma_start(out=vt, in_=vsp.ap()[st * P:(st + 1) * P, :])
                    for e in range(NE):
                        nc.tensor.matmul(pss[e], vt[:, e * P:(e + 1) * P],
                                         expT[:, st, tb * FD:(tb + 1) * FD],
                                         start=(st == 0), stop=(st == NS - 1))
                for e in range(NE):
                    nc.vector.tensor_copy(ctxT[:, e, tb * FD:(tb + 1) * FD], pss[e])
        expT_pool.release()

        # ---- P6: out projection + residual + LayerNorm ----
        with (
            tc.tile_pool(name="wo", bufs=1, side="right") as wop,
            tc.tile_pool(name="p6c", bufs=1, side="right") as p6c,
            tc.tile_pool(name="p6res", bufs=2, side="right") as resp,
            tc.tile_pool(name="p6y", bufs=2, side="right") as yp,
            tc.tile_pool(name="p6ln", bufs=4, side="right") as lnp,
            tc.tile_pool(name="p6out", bufs=2, side="right") as outp,
            tc.tile_pool(name="p6mm", bufs=3, space="PSUM") as mmp,
        ):
            wo_sb = _load_weight(nc, wop, wot)
            bo_sb = p6c.tile([P, E], f32)
            nc.gpsimd.dma_start(out=bo_sb, in_=bo2.ap().partition_broadcast(P))
            if apply_gb:
                gam_sb = p6c.tile([P, E], f32)
                nc.gpsimd.dma_start(out=gam_sb, in_=gam.ap().partition_broadcast(P))
                bet_sb = p6c.tile([P, E], f32)
                nc.gpsimd.dma_start(out=bet_sb, in_=bet.ap().partition_broadcast(P))
            for tt in range(NT):
                y = yp.tile([P, E], f32)
                for gc in range(E // FD):
                    ps = mmp.tile([P, FD], f32)
                    for j in range(NE):
                        nc.tensor.matmul(ps, ctxT[:, j, tt * P:(tt + 1) * P],
                                         wo_sb[:, j, gc * FD:(gc + 1) * FD],
                                         start=(j == 0), stop=(j == NE - 1))
                    nc.vector.tensor_scalar(
                        out=y[:, gc * FD:(gc + 1) * FD], in0=ps,
                        scalar1=recip_t[:, tt:tt + 1], scalar2=None, op0=ALU.mult)
                res = resp.tile([P, E], f32)
                nc.scalar.dma_start(out=res, in_=xq.ap()[tt * P:(tt + 1) * P, :])
                nc.vector.tensor_add(y, y, bo_sb)
                nc.vector.tensor_add(y, y, res)
                stats = lnp.tile([P, 2, 6], f32)
                nc.vector.bn_stats(stats[:, 0, :], y[:, 0:FD])
                nc.vector.bn_stats(stats[:, 1, :], y[:, FD:E])
                mv = lnp.tile([P, 2], f32)
                nc.vector.bn_aggr(mv, stats)
                rstd = lnp.tile([P, 1], f32)
                nc.scalar.activation(rstd, mv[:, 1:2], AF.Sqrt, bias=eps_t)
                nc.vector.reciprocal(rstd, rstd)
                o = outp.tile([P, E], f32)
                nc.vector.tensor_scalar(out=o, in0=y, scalar1=mv[:, 0:1],
                                        scalar2=rstd, op0=ALU.subtract, op1=ALU.mult)
                if apply_gb:
                    nc.vector.tensor_mul(o, o, gam_sb)
                    nc.vector.tensor_add(o, o, bet_sb)
                nc.scalar.dma_start(out=out.ap()[tt * P:(tt + 1) * P, :], in_=o)

        ctxT_pool.release()
        consts.release()

    nc.compile()
    return nc


def kernel(query, key, value, Wq, bq, Wk, bk, Wv, bv, Wo, bo, gamma, beta):
    query = np.asarray(query, dtype=np.float32)
    key = np.asarray(key, dtype=np.float32)
    value = np.asarray(value, dtype=np.float32)
    Wq = np.asarray(Wq, dtype=np.float32)
    bq = np.asarray(bq, dtype=np.float32)
    Wv = np.asarray(Wv, dtype=np.float32)
    bv = np.asarray(bv, dtype=np.float32)
    Wk = np.asarray(Wk, dtype=np.float32)
    Wo = np.asarray(Wo, dtype=np.float32)
    bo = np.asarray(bo, dtype=np.float32)
    gamma = np.asarray(gamma, dtype=np.float32)
    beta = np.asarray(beta, dtype=np.float32)

    scale = np.float32(1.0) / np.float32(np.sqrt(np.float32(E)))
    wqt = np.ascontiguousarray(Wq.T) * scale
    wkt = np.ascontiguousarray(Wk.T)
    wvt = np.ascontiguousarray(Wv.T)
    wot = np.ascontiguousarray(Wo.T)
    bq2 = np.ascontiguousarray((bq * scale).reshape(NE, P).T)
    bo2 = (bo + Wo @ bv).astype(np.float32)
    apply_gb = not (np.all(gamma == 1.0) and np.all(beta == 0.0))

    if apply_gb not in _cache:
        _cache[apply_gb] = _build(apply_gb)
    nc = _cache[apply_gb]

    def _split(x):
        hi = x.astype(BF16)
        lo = (x - hi.astype(np.float32)).astype(BF16)
        return hi, lo

    q_hi, q_lo = _split(query)
    k_hi, k_lo = _split(key)
    v_hi, v_lo = _split(value)

    in_maps = []
    for c in range(8):
        b, h = c // 2, c % 2
        m = {
            "xq": np.ascontiguousarray(query[b, h * T:(h + 1) * T]),
            "xq_hi": np.ascontiguousarray(q_hi[b, h * T:(h + 1) * T]),
            "xq_lo": np.ascontiguousarray(q_lo[b, h * T:(h + 1) * T]),
            "xk_hi": k_hi[b], "xk_lo": k_lo[b],
            "xv_hi": v_hi[b], "xv_lo": v_lo[b],
            "wqt": wqt, "wkt": wkt, "wvt": wvt, "wot": wot,
            "bq2": bq2, "bo2": bo2,
        }
        if apply_gb:
            m["gam"] = gamma
            m["bet"] = beta
        in_maps.append(m)

    global _saved_in_maps
    _saved_in_maps = in_maps
    res = run_bass_kernel_spmd(nc, in_maps, core_ids=list(range(8)))
    B = query.shape[0]
    full = np.empty((B, 2 * T, E), dtype=np.float32)
    for c in range(8):
        b, h = c // 2, c % 2
        full[b, h * T:(h + 1) * T] = res.results[c]["out"]
    return full


# revision 11
# speedup vs baseline: 1.1949x; 1.1949x over previous
"""Trainium2 Bass kernel for nn_MultiHeadAttention_5360119185803.

Full-d_model attention (no head split) + residual + LayerNorm, B=4, T=S=2048,
E=1024, fp32 in/out.

Sharding: 8 cores; core c owns batch b=c//2 and query rows
[(c%2)*1024, (c%2+1)*1024). Each core needs the full key/value of its batch
(K/V projection duplicated across the core pair) — no collectives.

Per-core device pipeline (all matmuls in float32r = TF32-like, full PE rate):
  P1  kT = (Wk.T).T @ xk.T           [f,s]
      (activations are transposed via 2-byte xbar DMA-transpose of a host-
      side bf16 hi/lo split, recombined hi+lo on DVE into f32r — keeps
      ~16-17 mantissa bits and zero PE time; PE transposes were 90us)
  P2  v  = xv @ Wv.T  -> spilled to DRAM (SBUF pressure), bias bv folded into
      bo' = bo + Wo@bv on host (attn rows sum to 1)
  P3  qT = (Wq.T/32).T @ xq.T + bq/32  [f,t]  (1/sqrt(E) folded into Wq, bq)
  P4  scoresT[s,t] = kT.T @ qT  (PSUM) -> expT = exp(scoresT)  (ACT, no
      max-subtraction: |scores/32| <~ 6 so exp is safe in fp32; bk dropped
      entirely — it shifts scores by a per-t constant, softmax-invariant)
      rowsum[1,t] = ones.T @ expT (PE), redistributed to [128,8] per-partition
  P5  ctxT[e',t] = sum_s v[s,e'] * expT[s,t]  (8 PSUM banks per t-half)
  P6  out[t,g] = (ctxT.T @ Wo.T) * (1/rowsum)[t] + bo'+ residual; LayerNorm
      over g on DVE (bn_stats/bn_aggr); gamma/beta applied only if non-trivial.

kernel() is self-contained: host prep = shard + weight transposes/scale folds.
"""

import sys

sys.path.insert(0, "/opt/trn_rl_repo")

import numpy as np

import ml_dtypes

import concourse.bacc as bacc
import concourse.bass as bass
import concourse.tile as tile
from concourse import mybir
from concourse.bass_utils import run_bass_kernel_spmd

BF16 = ml_dtypes.bfloat16

P = 128
E = 1024          # d_model
S = 2048          # kv seq len per batch
T = 1024          # query rows per core
NE = E // P       # 8 chunks of contraction dim
NT = T // P       # 8 t tiles
NS = S // P       # 16 s tiles
FD = 512          # matmul moving free dim / PSUM bank
NBLK_S = S // FD  # 4 s blocks
NBLK_T = T // FD  # 2 t blocks

f32 = mybir.dt.float32
f32r = mybir.dt.float32r
bf16 = mybir.dt.bfloat16
AF = mybir.ActivationFunctionType
ALU = mybir.AluOpType

_cache = {}


def _load_weight(nc, pool, dram):
    """[E, x] fp32 DRAM -> [128, NE, x] f32r SBUF (gpsimd DMA casts+rounds)."""
    w = pool.tile([P, NE, E], f32r)
    v = dram.ap().rearrange("(j p) f -> j p f", p=P)
    for j in range(NE):
        nc.gpsimd.dma_start(out=w[:, j, :], in_=v[j])
    return w


def _load_xt_block(nc, xt_blk, x_hi, x_lo, row0, nrows, hilo_pool):
    """xt_blk[:, j, :] (f32r [128, NE, nrows]) = x[row0:row0+nrows,
    j*128:(j+1)*128].T via two 2-byte xbar DMA-transposes + DVE hi+lo add.
    nc.sync is reserved for transposed DMAs (xbar-mode transitions serialize,
    so plain DMAs ride nc.scalar/gpsimd instead)."""
    for j in range(NE):
        eng = nc.sync if j % 2 == 0 else nc.scalar
        th = hilo_pool.tile([P, nrows], bf16, name=f"th{row0}_{j}", tag=f"th{j%2}")
        eng.dma_start(
            out=th, in_=x_hi.ap()[row0:row0 + nrows, j * P:(j + 1) * P],
            transpose=True)
        tl = hilo_pool.tile([P, nrows], bf16, name=f"tl{row0}_{j}", tag=f"tl{j%2}")
        eng.dma_start(
            out=tl, in_=x_lo.ap()[row0:row0 + nrows, j * P:(j + 1) * P],
            transpose=True)
        nc.vector.tensor_tensor(xt_blk[:, j, :], th, tl, ALU.add)


def _build(apply_gb):
    nc = bacc.Bacc("TRN2", target_bir_lowering=False, debug=False, num_devices=8)

    xq = nc.dram_tensor("xq", [T, E], f32, kind="ExternalInput")
    xq_hi = nc.dram_tensor("xq_hi", [T, E], bf16, kind="ExternalInput")
    xq_lo = nc.dram_tensor("xq_lo", [T, E], bf16, kind="ExternalInput")
    xk_hi = nc.dram_tensor("xk_hi", [S, E], bf16, kind="ExternalInput")
    xk_lo = nc.dram_tensor("xk_lo", [S, E], bf16, kind="ExternalInput")
    xv_hi = nc.dram_tensor("xv_hi", [S, E], bf16, kind="ExternalInput")
    xv_lo = nc.dram_tensor("xv_lo", [S, E], bf16, kind="ExternalInput")
    wqt = nc.dram_tensor("wqt", [E, E], f32, kind="ExternalInput")  # Wq.T/32 [e,f]
    wkt = nc.dram_tensor("wkt", [E, E], f32, kind="ExternalInput")  # Wk.T   [e,f]
    wvt = nc.dram_tensor("wvt", [E, E], f32, kind="ExternalInput")  # Wv.T   [e,e']
    wot = nc.dram_tensor("wot", [E, E], f32, kind="ExternalInput")  # Wo.T   [e',g]
    bq2 = nc.dram_tensor("bq2", [P, NE], f32, kind="ExternalInput")  # bq/32 tiled
    bo2 = nc.dram_tensor("bo2", [E], f32, kind="ExternalInput")      # bo + Wo@bv
    if apply_gb:
        gam = nc.dram_tensor("gam", [E], f32, kind="ExternalInput")
        bet = nc.dram_tensor("bet", [E], f32, kind="ExternalInput")
    out = nc.dram_tensor("out", [T, E], f32, kind="ExternalOutput")
    vsp = nc.dram_tensor("v_spill", [S, E], f32r)
    rs_dram = nc.dram_tensor("rs_scratch", [T], f32)

    with tile.TileContext(nc) as tc:
        consts = tc.alloc_tile_pool(name="consts", bufs=1, side="left")
        eps_t = consts.tile([P, 1], f32)
        nc.vector.memset(eps_t, 1e-6)
        ones_f = consts.tile([P, 1], f32)
        nc.vector.memset(ones_f, 1.0)
        ones_r = consts.tile([P, 1], f32r)
        nc.vector.tensor_copy(ones_r, ones_f)
        recip_t = consts.tile([P, NT], f32)

        kT_pool = tc.alloc_tile_pool(name="kT", bufs=1, side="left")
        kT = kT_pool.tile([P, NE, S], f32r)  # [f, fchunk, s] 8MB
        qT_pool = tc.alloc_tile_pool(name="qT", bufs=1, side="left")
        qT = qT_pool.tile([P, NE, T], f32r)  # [f, fchunk, t] 4MB
        identp = tc.alloc_tile_pool(name="identp", bufs=1, side="left")
        bq_sb = identp.tile([P, NE], f32)
        nc.gpsimd.dma_start(out=bq_sb, in_=bq2.ap())

        # ---- P1: kT projection ----
        with (
            tc.tile_pool(name="wk", bufs=1) as wkp,
            tc.tile_pool(name="p1hl", bufs=3) as hlp,
            tc.tile_pool(name="p1xt", bufs=2) as xtp,
            tc.tile_pool(name="p1mm", bufs=3, space="PSUM") as mmp,
        ):
            wk_sb = _load_weight(nc, wkp, wkt)
            for sb in range(NBLK_S):
                xt_blk = xtp.tile([P, NE, FD], f32r)
                _load_xt_block(nc, xt_blk, xk_hi, xk_lo, sb * FD, FD, hlp)
                for ft in range(NE):
                    ps = mmp.tile([P, FD], f32)
                    for j in range(NE):
                        nc.tensor.matmul(ps, wk_sb[:, j, ft * P:(ft + 1) * P],
                                         xt_blk[:, j, :],
                                         start=(j == 0), stop=(j == NE - 1))
                    nc.vector.tensor_copy(kT[:, ft, sb * FD:(sb + 1) * FD], ps)

        # ---- P2: v projection -> DRAM spill ----
        with (
            tc.tile_pool(name="wv", bufs=1) as wvp,
            tc.tile_pool(name="p2hl", bufs=3) as hlp,
            tc.tile_pool(name="p2xt", bufs=2) as xtp,
            tc.tile_pool(name="p2mm", bufs=3, space="PSUM") as mmp,
            tc.tile_pool(name="p2ev", bufs=3) as evp,
        ):
            wv_sb = _load_weight(nc, wvp, wvt)
            for sb in range(NBLK_S):
                xt_blk = xtp.tile([P, NE, FD], f32r)
                _load_xt_block(nc, xt_blk, xv_hi, xv_lo, sb * FD, FD, hlp)
                for ss in range(FD // P):
                    ev = evp.tile([P, E], f32r)
                    for ec in range(E // FD):
                        ps = mmp.tile([P, FD], f32)
                        for j in range(NE):
                            nc.tensor.matmul(ps, xt_blk[:, j, ss * P:(ss + 1) * P],
                                             wv_sb[:, j, ec * FD:(ec + 1) * FD],
                                             start=(j == 0), stop=(j == NE - 1))
                        nc.vector.tensor_copy(ev[:, ec * FD:(ec + 1) * FD], ps)
                    r0 = sb * FD + ss * P
                    nc.gpsimd.dma_start(out=vsp.ap()[r0:r0 + P, :], in_=ev)

        # ---- P3: qT projection (+bq/32) ----
        with (
            tc.tile_pool(name="wq", bufs=1) as wqp,
            tc.tile_pool(name="p3hl", bufs=3) as hlp,
            tc.tile_pool(name="p3xt", bufs=2) as xtp,
            tc.tile_pool(name="p3mm", bufs=3, space="PSUM") as mmp,
        ):
            wq_sb = _load_weight(nc, wqp, wqt)
            for tb in range(NBLK_T):
                xt_blk = xtp.tile([P, NE, FD], f32r)
                _load_xt_block(nc, xt_blk, xq_hi, xq_lo, tb * FD, FD, hlp)
                for ft in range(NE):
                    ps = mmp.tile([P, FD], f32)
                    for j in range(NE):
                        nc.tensor.matmul(ps, wq_sb[:, j, ft * P:(ft + 1) * P],
                                         xt_blk[:, j, :],
                                         start=(j == 0), stop=(j == NE - 1))
                    nc.vector.tensor_scalar(
                        out=qT[:, ft, tb * FD:(tb + 1) * FD], in0=ps,
                        scalar1=bq_sb[:, ft:ft + 1], scalar2=None, op0=ALU.add)

        identp.release()

        # ---- P4: scoresT -> expT; rowsum -> recip ----
        ctxT_pool = tc.alloc_tile_pool(name="ctxT", bufs=1, side="right")
        ctxT = ctxT_pool.tile([P, NE, T], f32r)  # [e', echunk, t] 4MB
        expT_pool = tc.alloc_tile_pool(name="expT", bufs=1, side="right")
        expT = expT_pool.tile([P, NS, T], f32r)  # [s, stile, t] 8MB
        with tc.tile_pool(name="p4mm", bufs=4, space="PSUM") as mmp:
            for st in range(NS):
                for tb in range(NBLK_T):
                    ps = mmp.tile([P, FD], f32)
                    for j in range(NE):
                        nc.tensor.matmul(ps, kT[:, j, st * P:(st + 1) * P],
                                         qT[:, j, tb * FD:(tb + 1) * FD],
                                         start=(j == 0), stop=(j == NE - 1))
                    nc.scalar.activation(expT[:, st, tb * FD:(tb + 1) * FD], ps, AF.Exp)

        with (
            tc.tile_pool(name="p4rs", bufs=2, space="PSUM") as rsp,
            tc.tile_pool(name="p4rw", bufs=1, side="right") as rwp,
        ):
            rs_sb = rwp.tile([1, T], f32)
            for tb in range(NBLK_T):
                ps = rsp.tile([P, FD], f32)
                for st in range(NS):
                    nc.tensor.matmul(ps[0:1, :], ones_r[:, 0:1],
                                     expT[:, st, tb * FD:(tb + 1) * FD],
                                     start=(st == 0), stop=(st == NS - 1))
                nc.vector.tensor_copy(rs_sb[0:1, tb * FD:(tb + 1) * FD], ps[0:1, :])
            nc.gpsimd.dma_start(out=rs_dram.ap(), in_=rs_sb[0:1, :])
            rsT = rwp.tile([P, NT], f32)
            nc.gpsimd.dma_start(out=rsT, in_=rs_dram.ap().rearrange("(j p) -> p j", p=P))
            nc.vector.reciprocal(recip_t, rsT)

        qT_pool.release()
        kT_pool.release()

        # ---- P5: ctxT ----
        with (
            tc.tile_pool(name="p5v", bufs=3, side="right") as vp,
            tc.tile_pool(name="p5mm", bufs=1, space="PSUM") as mmp,
        ):
            for tb in range(NBLK_T):
                pss = [mmp.tile([P, FD], f32, name=f"ctxps{tb}_{e}",
                                tag=f"ctxps{e}") for e in range(NE)]
                for st in range(NS):
                    vt = vp.tile([P, E], f32r)
                    nc.scalar.dma_start(out=vt, in_=vsp.ap()[st * P:(st + 1) * P, :])
                    for e in range(NE):
                        nc.tensor.matmul(pss[e], vt[:, e * P:(e + 1) * P],
                                         expT[:, st, tb * FD:(tb + 1) * FD],
                                         start=(st == 0), stop=(st == NS - 1))
                for e in range(NE):
                    nc.vector.tensor_copy(ctxT[:, e, tb * FD:(tb + 1) * FD], pss[e])
        expT_pool.release()

        # ---- P6: out projection + residual + LayerNorm ----
        with (
            tc.tile_pool(name="wo", bufs=1, side="right") as wop,
            tc.tile_pool(name="p6c", bufs=1, side="right") as p6c,
            tc.tile_pool(name="p6res", bufs=2, side="right") as resp,
            tc.tile_pool(name="p6y", bufs=2, side="right") as yp,
            tc.tile_pool(name="p6ln", bufs=4, side="right") as lnp,
            tc.tile_pool(name="p6out", bufs=2, side="right") as outp,
            tc.tile_pool(name="p6mm", bufs=3, space="PSUM") as mmp,
        ):
            wo_sb = _load_weight(nc, wop, wot)
            bo_sb = p6c.tile([P, E], f32)
            nc.gpsimd.dma_start(out=bo_sb, in_=bo2.ap().partition_broadcast(P))
            if apply_gb:
                gam_sb = p6c.tile([P, E], f32)
                nc.gpsimd.dma_start(out=gam_sb, in_=gam.ap().partition_broadcast(P))
                bet_sb = p6c.tile([P, E], f32)
                nc.gpsimd.dma_start(out=bet_sb, in_=bet.ap().partition_broadcast(P))
            for tt in range(NT):
                y = yp.tile([P, E], f32)
                for gc in range(E // FD):
                    ps = mmp.tile([P, FD], f32)
                    for j in range(NE):
                        nc.tensor.matmul(ps, ctxT[:, j, tt * P:(tt + 1) * P],
                                         wo_sb[:, j, gc * FD:(gc + 1) * FD],
                                         start=(j == 0), stop=(j == NE - 1))
                    nc.vector.tensor_scalar(
                        out=y[:, gc * FD:(gc + 1) * FD], in0=ps,
                        scalar1=recip_t[:, tt:tt + 1], scalar2=None, op0=ALU.mult)
                res = resp.tile([P, E], f32)
                nc.scalar.dma_start(out=res, in_=xq.ap()[tt * P:(tt + 1) * P, :])
                nc.vector.tensor_add(y, y, bo_sb)
                nc.vector.tensor_add(y, y, res)
                stats = lnp.tile([P, 2, 6], f32)
                nc.vector.bn_stats(stats[:, 0, :], y[:, 0:FD])
                nc.vector.bn_stats(stats[:, 1, :], y[:, FD:E])
                mv = lnp.tile([P, 2], f32)
                nc.vector.bn_aggr(mv, stats)
                rstd = lnp.tile([P, 1], f32)
                nc.scalar.activation(rstd, mv[:, 1:2], AF.Sqrt, bias=eps_t)
                nc.vector.reciprocal(rstd, rstd)
                o = outp.tile([P, E], f32)
                nc.vector.tensor_scalar(out=o, in0=y, scalar1=mv[:, 0:1],
                                        scalar2=rstd, op0=ALU.subtract, op1=ALU.mult)
                if apply_gb:
                    nc.vector.tensor_mul(o, o, gam_sb)
                    nc.vector.tensor_add(o, o, bet_sb)
                nc.scalar.dma_start(out=out.ap()[tt * P:(tt + 1) * P, :], in_=o)

        ctxT_pool.release()
        consts.release()

    nc.compile()
    return nc


def kernel(query, key, value, Wq, bq, Wk, bk, Wv, bv, Wo, bo, gamma, beta):
    query = np.asarray(query, dtype=np.float32)
    key = np.asarray(key, dtype=np.float32)
    value = np.asarray(value, dtype=np.float32)
    Wq = np.asarray(Wq, dtype=np.float32)
    bq = np.asarray(bq, dtype=np.float32)
    Wv = np.asarray(Wv, dtype=np.float32)
    bv = np.asarray(bv, dtype=np.float32)
    Wk = np.asarray(Wk, dtype=np.float32)
    Wo = np.asarray(Wo, dtype=np.float32)
    bo = np.asarray(bo, dtype=np.float32)
    gamma = np.asarray(gamma, dtype=np.float32)
    beta = np.asarray(beta, dtype=np.float32)

    scale = np.float32(1.0) / np.float32(np.sqrt(np.float32(E)))
    wqt = np.ascontiguousarray(Wq.T) * scale
    wkt = np.ascontiguousarray(Wk.T)
    wvt = np.ascontiguousarray(Wv.T)
    wot = np.ascontiguousarray(Wo.T)
    bq2 = np.ascontiguousarray((bq * scale).reshape(NE, P).T)
    bo2 = (bo + Wo @ bv).astype(np.float32)
    apply_gb = not (np.all(gamma == 1.0) and np.all(beta == 0.0))

    if apply_gb not in _cache:
        _cache[apply_gb] = _build(apply_gb)
    nc = _cache[apply_gb]

    def _split(x):
        hi = x.astype(BF16)
        lo = (x - hi.astype(np.float32)).astype(BF16)
        return hi, lo

    q_hi, q_lo = _split(query)
    k_hi, k_lo = _split(key)
    v_hi, v_lo = _split(value)

    in_maps = []
    for c in range(8):
        b, h = c // 2, c % 2
        m = {
            "xq": np.ascontiguousarray(query[b, h * T:(h + 1) * T]),
            "xq_hi": np.ascontiguousarray(q_hi[b, h * T:(h + 1) * T]),
            "xq_lo": np.ascontiguousarray(q_lo[b, h * T:(h + 1) * T]),
            "xk_hi": k_hi[b], "xk_lo": k_lo[b],
            "xv_hi": v_hi[b], "xv_lo": v_lo[b],
            "wqt": wqt, "wkt": wkt, "wvt": wvt, "wot": wot,
            "bq2": bq2, "bo2": bo2,
        }
        if apply_gb:
            m["gam"] = gamma
            m["bet"] = beta
        in_maps.append(m)

    global _saved_in_maps
    _saved_in_maps = in_maps
    res = run_bass_kernel_spmd(nc, in_maps, core_ids=list(range(8)))
    B = query.shape[0]
    full = np.empty((B, 2 * T, E), dtype=np.float32)
    for c in range(8):
        b, h = c // 2, c % 2
        full[b, h * T:(h + 1) * T] = res.results[c]["out"]
    return full


# revision 12
# speedup vs baseline: 1.2368x; 1.0350x over previous
"""Trainium2 Bass kernel for nn_MultiHeadAttention_5360119185803.

Full-d_model attention (no head split) + residual + LayerNorm, B=4, T=S=2048,
E=1024, fp32 in/out.

Sharding: 8 cores; core c owns batch b=c//2 and query rows
[(c%2)*1024, (c%2+1)*1024). K/V projections are split across the core pair
(each projects 1024 of the 2048 kv rows) and exchanged with a pair-wise
AllGather — halves the duplicated projection + transpose work.

Per-core device pipeline (all matmuls in float32r = TF32-like, full PE rate):
  P1  kT_half = (Wk.T).T @ xkh.T  [f, s_local]  -> DRAM, AllGather(pair)
      (activation transposes on PE via identity-matmul, fp32)
  P2  v_half  = xvh @ Wv.T        -> DRAM, AllGather(pair); bias bv folded
      into bo' = bo + Wo@bv on host (attn rows sum to 1)
  P3  qT = (Wq.T/32).T @ xq.T + bq/32  [f,t]  (1/sqrt(E) folded into Wq, bq)
  P4  load gathered kT; scoresT[s,t] = kT.T @ qT (PSUM) -> expT = exp(scoresT)
      (ACT; no max-subtraction: |scores/32| <~ 6 so exp is fp32-safe; bk
      dropped entirely — it shifts scores by a per-t constant, softmax-
      invariant). rowsum[1,t] = ones.T @ expT (PE), redistributed via DRAM.
  P5  ctxT[e',t] = sum_s v[s,e'] * expT[s,t]  (8 PSUM banks per t-half,
      v streamed from the gathered DRAM copy)
  P6  out[t,g] = (ctxT.T @ Wo.T) * (1/rowsum)[t] + bo' + residual; LayerNorm
      over g (bn_stats/bn_aggr on DVE, psum evict on ACT, bo-add on GpSimd);
      gamma/beta applied only if non-trivial.

kernel() is self-contained: host prep = shard + weight transposes/scale folds.
"""

import sys

sys.path.insert(0, "/opt/trn_rl_repo")

import numpy as np

import concourse.bacc as bacc
import concourse.bass as bass
import concourse.tile as tile
from concourse import mybir
from concourse.bass_utils import run_bass_kernel_spmd
from concourse.masks import make_identity

P = 128
E = 1024          # d_model
S = 2048          # kv seq len per batch
SH = S // 2       # kv rows projected locally
T = 1024          # query rows per core
NE = E // P       # 8 chunks of contraction dim
NT = T // P       # 8 t tiles
NS = S // P       # 16 s tiles
FD = 512          # matmul moving free dim / PSUM bank
NBLK_T = T // FD  # 2 blocks of 512

f32 = mybir.dt.float32
f32r = mybir.dt.float32r
AF = mybir.ActivationFunctionType
ALU = mybir.AluOpType
GROUPS = [[0, 1], [2, 3], [4, 5], [6, 7]]

_cache = {}


def _load_weight(nc, pool, dram):
    """[E, x] fp32 DRAM -> [128, NE, x] f32r SBUF (gpsimd DMA casts+rounds)."""
    w = pool.tile([P, NE, E], f32r)
    v = dram.ap().rearrange("(j p) f -> j p f", p=P)
    for j in range(NE):
        nc.gpsimd.dma_start(out=w[:, j, :], in_=v[j])
    return w


def _transpose_block(nc, xt_blk, x_dram, row0, nrows, nat_pool, tp_psum, ident):
    """xt_blk[:, j, :] (f32r [128, NE, nrows]) = x[row0:row0+nrows,
    j*128:(j+1)*128].T via PE identity-transpose + DVE psum evict."""
    for ss in range(nrows // P):
        nat = nat_pool.tile([P, E], f32)
        nc.sync.dma_start(out=nat, in_=x_dram.ap()[row0 + ss * P: row0 + (ss + 1) * P, :])
        for j in range(NE):
            ps = tp_psum.tile([P, P], f32)
            nc.tensor.transpose(ps, nat[:, j * P:(j + 1) * P], ident)
            nc.vector.tensor_copy(xt_blk[:, j, ss * P:(ss + 1) * P], ps)


def _build(apply_gb):
    nc = bacc.Bacc("TRN2", target_bir_lowering=False, debug=False, num_devices=8)

    xq = nc.dram_tensor("xq", [T, E], f32, kind="ExternalInput")
    xkh = nc.dram_tensor("xkh", [SH, E], f32, kind="ExternalInput")
    xvh = nc.dram_tensor("xvh", [SH, E], f32, kind="ExternalInput")
    wqt = nc.dram_tensor("wqt", [E, E], f32, kind="ExternalInput")  # Wq.T/32 [e,f]
    wkt = nc.dram_tensor("wkt", [E, E], f32, kind="ExternalInput")  # Wk.T   [e,f]
    wvt = nc.dram_tensor("wvt", [E, E], f32, kind="ExternalInput")  # Wv.T   [e,e']
    wot = nc.dram_tensor("wot", [E, E], f32, kind="ExternalInput")  # Wo.T   [e',g]
    bq2 = nc.dram_tensor("bq2", [P, NE], f32, kind="ExternalInput")  # bq/32 tiled
    bo2 = nc.dram_tensor("bo2", [E], f32, kind="ExternalInput")      # bo + Wo@bv
    if apply_gb:
        gam = nc.dram_tensor("gam", [E], f32, kind="ExternalInput")
        bet = nc.dram_tensor("bet", [E], f32, kind="ExternalInput")
    out = nc.dram_tensor("out", [T, E], f32, kind="ExternalOutput")

    kth = nc.dram_tensor("kth", [P, NE, SH], f32r)       # local kT half
    ktg = nc.dram_tensor("ktg", [2, P, NE, SH], f32r)    # gathered kT
    vh = nc.dram_tensor("vh", [SH, E], f32r)             # local v half
    vg = nc.dram_tensor("vg", [2, SH, E], f32r)          # gathered v
    rs_dram = nc.dram_tensor("rs_scratch", [T], f32)

    with tile.TileContext(nc) as tc:
        consts = tc.alloc_tile_pool(name="consts", bufs=1, side="left")
        eps_t = consts.tile([P, 1], f32)
        nc.vector.memset(eps_t, 1e-6)
        ones_f = consts.tile([P, 1], f32)
        nc.vector.memset(ones_f, 1.0)
        ones_r = consts.tile([P, 1], f32r)
        nc.vector.tensor_copy(ones_r, ones_f)
        recip_t = consts.tile([P, NT], f32)

        kT_pool = tc.alloc_tile_pool(name="kT", bufs=1, side="left")
        kT = kT_pool.tile([P, NE, S], f32r)  # [f, fchunk, s] 8MB
        qT_pool = tc.alloc_tile_pool(name="qT", bufs=1, side="left")
        qT = qT_pool.tile([P, NE, T], f32r)  # [f, fchunk, t] 4MB
        identp = tc.alloc_tile_pool(name="identp", bufs=1, side="left")
        ident = identp.tile([P, P], f32)
        make_identity(nc, ident)
        bq_sb = identp.tile([P, NE], f32)
        nc.sync.dma_start(out=bq_sb, in_=bq2.ap())

        # ---- P1: kT half projection -> DRAM -> pair AllGather ----
        with (
            tc.tile_pool(name="wk", bufs=1) as wkp,
            tc.tile_pool(name="p1nat", bufs=3) as natp,
            tc.tile_pool(name="p1xt", bufs=2) as xtp,
            tc.tile_pool(name="p1ev", bufs=3) as evp,
            tc.tile_pool(name="p1tp", bufs=4, space="PSUM") as tpp,
            tc.tile_pool(name="p1mm", bufs=4, space="PSUM") as mmp,
        ):
            wk_sb = _load_weight(nc, wkp, wkt)
            for sb in range(SH // FD):
                xt_blk = xtp.tile([P, NE, FD], f32r)
                _transpose_block(nc, xt_blk, xkh, sb * FD, FD, natp, tpp, ident)
                for ft in range(NE):
                    ps = mmp.tile([P, FD], f32)
                    for j in range(NE):
                        nc.tensor.matmul(ps, wk_sb[:, j, ft * P:(ft + 1) * P],
                                         xt_blk[:, j, :],
                                         start=(j == 0), stop=(j == NE - 1))
                    ev = evp.tile([P, FD], f32r)
                    nc.vector.tensor_copy(ev, ps)
                    nc.scalar.dma_start(
                        out=kth.ap()[:, ft, sb * FD:(sb + 1) * FD], in_=ev)
            nc.gpsimd.collective_compute(
                "AllGather", ALU.bypass, replica_groups=GROUPS,
                ins=[kth.ap()], outs=[ktg.ap()])

        # ---- P2: v half projection -> DRAM -> pair AllGather ----
        with (
            tc.tile_pool(name="wv", bufs=1) as wvp,
            tc.tile_pool(name="p2nat", bufs=3) as natp,
            tc.tile_pool(name="p2xt", bufs=2) as xtp,
            tc.tile_pool(name="p2ev", bufs=3) as evp,
            tc.tile_pool(name="p2tp", bufs=4, space="PSUM") as tpp,
            tc.tile_pool(name="p2mm", bufs=4, space="PSUM") as mmp,
        ):
            wv_sb = _load_weight(nc, wvp, wvt)
            for sb in range(SH // FD):
                xt_blk = xtp.tile([P, NE, FD], f32r)
                _transpose_block(nc, xt_blk, xvh, sb * FD, FD, natp, tpp, ident)
                for ss in range(FD // P):
                    ev = evp.tile([P, E], f32r)
                    for ec in range(E // FD):
                        ps = mmp.tile([P, FD], f32)
                        for j in range(NE):
                            nc.tensor.matmul(ps, xt_blk[:, j, ss * P:(ss + 1) * P],
                                             wv_sb[:, j, ec * FD:(ec + 1) * FD],
                                             start=(j == 0), stop=(j == NE - 1))
                        nc.vector.tensor_copy(ev[:, ec * FD:(ec + 1) * FD], ps)
                    r0 = sb * FD + ss * P
                    nc.scalar.dma_start(out=vh.ap()[r0:r0 + P, :], in_=ev)
            nc.gpsimd.collective_compute(
                "AllGather", ALU.bypass, replica_groups=GROUPS,
                ins=[vh.ap()], outs=[vg.ap()])

        # ---- P3: qT projection (+bq/32) ----
        with (
            tc.tile_pool(name="wq", bufs=1) as wqp,
            tc.tile_pool(name="p3nat", bufs=3) as natp,
            tc.tile_pool(name="p3xt", bufs=2) as xtp,
            tc.tile_pool(name="p3tp", bufs=4, space="PSUM") as tpp,
            tc.tile_pool(name="p3mm", bufs=4, space="PSUM") as mmp,
        ):
            wq_sb = _load_weight(nc, wqp, wqt)
            for tb in range(NBLK_T):
                xt_blk = xtp.tile([P, NE, FD], f32r)
                _transpose_block(nc, xt_blk, xq, tb * FD, FD, natp, tpp, ident)
                for ft in range(NE):
                    ps = mmp.tile([P, FD], f32)
                    for j in range(NE):
                        nc.tensor.matmul(ps, wq_sb[:, j, ft * P:(ft + 1) * P],
                                         xt_blk[:, j, :],
                                         start=(j == 0), stop=(j == NE - 1))
                    nc.vector.tensor_scalar(
                        out=qT[:, ft, tb * FD:(tb + 1) * FD], in0=ps,
                        scalar1=bq_sb[:, ft:ft + 1], scalar2=None, op0=ALU.add)
        identp.release()

        # ---- P4: load gathered kT; scoresT -> expT; rowsum -> recip ----
        for r in range(2):
            for j in range(NE):
                nc.scalar.dma_start(out=kT[:, j, r * SH:(r + 1) * SH],
                                    in_=ktg.ap()[r, :, j, :])

        ctxT_pool = tc.alloc_tile_pool(name="ctxT", bufs=1, side="right")
        ctxT = ctxT_pool.tile([P, NE, T], f32r)  # [e', echunk, t] 4MB
        expT_pool = tc.alloc_tile_pool(name="expT", bufs=1, side="right")
        expT = expT_pool.tile([P, NS, T], f32r)  # [s, stile, t] 8MB
        with tc.tile_pool(name="p4mm", bufs=4, space="PSUM") as mmp:
            for st in range(NS):
                for tb in range(NBLK_T):
                    ps = mmp.tile([P, FD], f32)
                    for j in range(NE):
                        nc.tensor.matmul(ps, kT[:, j, st * P:(st + 1) * P],
                                         qT[:, j, tb * FD:(tb + 1) * FD],
                                         start=(j == 0), stop=(j == NE - 1))
                    nc.scalar.activation(expT[:, st, tb * FD:(tb + 1) * FD], ps, AF.Exp)

        with (
            tc.tile_pool(name="p4rs", bufs=2, space="PSUM") as rsp,
            tc.tile_pool(name="p4rw", bufs=1, side="right") as rwp,
        ):
            rs_sb = rwp.tile([1, T], f32)
            for tb in range(NBLK_T):
                ps = rsp.tile([P, FD], f32)
                for st in range(NS):
                    nc.tensor.matmul(ps[0:1, :], ones_r[:, 0:1],
                                     expT[:, st, tb * FD:(tb + 1) * FD],
                                     start=(st == 0), stop=(st == NS - 1))
                nc.vector.tensor_copy(rs_sb[0:1, tb * FD:(tb + 1) * FD], ps[0:1, :])
            nc.gpsimd.dma_start(out=rs_dram.ap(), in_=rs_sb[0:1, :])
            rsT = rwp.tile([P, NT], f32)
            nc.gpsimd.dma_start(out=rsT, in_=rs_dram.ap().rearrange("(j p) -> p j", p=P))
            nc.vector.reciprocal(recip_t, rsT)

        qT_pool.release()
        kT_pool.release()

        # ---- P5: ctxT ----
        vflat = vg.ap().rearrange("r s e -> (r s) e")
        with (
            tc.tile_pool(name="p5v", bufs=3, side="right") as vp,
            tc.tile_pool(name="p5mm", bufs=1, space="PSUM") as mmp,
        ):
            for tb in range(NBLK_T):
                pss = [mmp.tile([P, FD], f32, name=f"ctxps{tb}_{e}",
                                tag=f"ctxps{e}") for e in range(NE)]
                for st in range(NS):
                    vt = vp.tile([P, E], f32r)
                    nc.scalar.dma_start(out=vt, in_=vflat[st * P:(st + 1) * P, :])
                    for e in range(NE):
                        nc.tensor.matmul(pss[e], vt[:, e * P:(e + 1) * P],
                                         expT[:, st, tb * FD:(tb + 1) * FD],
                                         start=(st == 0), stop=(st == NS - 1))
                for e in range(NE):
                    nc.vector.tensor_copy(ctxT[:, e, tb * FD:(tb + 1) * FD], pss[e])
        expT_pool.release()

        # ---- P6: out projection + residual + LayerNorm ----
        with (
            tc.tile_pool(name="wo", bufs=1, side="right") as wop,
            tc.tile_pool(name="p6c", bufs=1, side="right") as p6c,
            tc.tile_pool(name="p6res", bufs=2, side="right") as resp,
            tc.tile_pool(name="p6y", bufs=2, side="right") as yp,
            tc.tile_pool(name="p6ln", bufs=4, side="right") as lnp,
            tc.tile_pool(name="p6out", bufs=2, side="right") as outp,
            tc.tile_pool(name="p6mm", bufs=3, space="PSUM") as mmp,
        ):
            wo_sb = _load_weight(nc, wop, wot)
            bo_sb = p6c.tile([P, E], f32)
            nc.gpsimd.dma_start(out=bo_sb, in_=bo2.ap().partition_broadcast(P))
            if apply_gb:
                gam_sb = p6c.tile([P, E], f32)
                nc.gpsimd.dma_start(out=gam_sb, in_=gam.ap().partition_broadcast(P))
                bet_sb = p6c.tile([P, E], f32)
                nc.gpsimd.dma_start(out=bet_sb, in_=bet.ap().partition_broadcast(P))
            for tt in range(NT):
                y = yp.tile([P, E], f32)
                for gc in range(E // FD):
                    ps = mmp.tile([P, FD], f32)
                    for j in range(NE):
                        nc.tensor.matmul(ps, ctxT[:, j, tt * P:(tt + 1) * P],
                                         wo_sb[:, j, gc * FD:(gc + 1) * FD],
                                         start=(j == 0), stop=(j == NE - 1))
                    # y = psum / rowsum  (ACT: Copy(in*scale))
                    nc.scalar.activation(y[:, gc * FD:(gc + 1) * FD], ps, AF.Copy,
                                         scale=recip_t[:, tt:tt + 1])
                res = resp.tile([P, E], f32)
                nc.sync.dma_start(out=res, in_=xq.ap()[tt * P:(tt + 1) * P, :])
                nc.gpsimd.tensor_tensor(y, y, bo_sb, ALU.add)
                nc.vector.tensor_add(y, y, res)
                stats = lnp.tile([P, 2, 6], f32)
                nc.vector.bn_stats(stats[:, 0, :], y[:, 0:FD])
                nc.vector.bn_stats(stats[:, 1, :], y[:, FD:E])
                mv = lnp.tile([P, 2], f32)
                nc.vector.bn_aggr(mv, stats)
                rstd = lnp.tile([P, 1], f32)
                nc.scalar.activation(rstd, mv[:, 1:2], AF.Sqrt, bias=eps_t)
                nc.vector.reciprocal(rstd, rstd)
                o = outp.tile([P, E], f32)
                nc.vector.tensor_scalar(out=o, in0=y, scalar1=mv[:, 0:1],
                                        scalar2=rstd, op0=ALU.subtract, op1=ALU.mult)
                if apply_gb:
                    nc.vector.tensor_mul(o, o, gam_sb)
                    nc.vector.tensor_add(o, o, bet_sb)
                nc.sync.dma_start(out=out.ap()[tt * P:(tt + 1) * P, :], in_=o)

        ctxT_pool.release()
        consts.release()

    nc.compile()
    return nc


def kernel(query, key, value, Wq, bq, Wk, bk, Wv, bv, Wo, bo, gamma, beta):
    query = np.asarray(query, dtype=np.float32)
    key = np.asarray(key, dtype=np.float32)
    value = np.asarray(value, dtype=np.float32)
    Wq = np.asarray(Wq, dtype=np.float32)
    bq = np.asarray(bq, dtype=np.float32)
    Wv = np.asarray(Wv, dtype=np.float32)
    bv = np.asarray(bv, dtype=np.float32)
    Wk = np.asarray(Wk, dtype=np.float32)
    Wo = np.asarray(Wo, dtype=np.float32)
    bo = np.asarray(bo, dtype=np.float32)
    gamma = np.asarray(gamma, dtype=np.float32)
    beta = np.asarray(beta, dtype=np.float32)

    scale = np.float32(1.0) / np.float32(np.sqrt(np.float32(E)))
    wqt = np.ascontiguousarray(Wq.T) * scale
    wkt = np.ascontiguousarray(Wk.T)
    wvt = np.ascontiguousarray(Wv.T)
    wot = np.ascontiguousarray(Wo.T)
    bq2 = np.ascontiguousarray((bq * scale).reshape(NE, P).T)
    bo2 = (bo + Wo @ bv).astype(np.float32)
    apply_gb = not (np.all(gamma == 1.0) and np.all(beta == 0.0))

    if apply_gb not in _cache:
        _cache[apply_gb] = _build(apply_gb)
    nc = _cache[apply_gb]

    in_maps = []
    for c in range(8):
        b, h = c // 2, c % 2
        m = {
            "xq": np.ascontiguousarray(query[b, h * T:(h + 1) * T]),
            "xkh": np.ascontiguousarray(key[b, h * SH:(h + 1) * SH]),
            "xvh": np.ascontiguousarray(value[b, h * SH:(h + 1) * SH]),
            "wqt": wqt, "wkt": wkt, "wvt": wvt, "wot": wot,
            "bq2": bq2, "bo2": bo2,
        }
        if apply_gb:
            m["gam"] = gamma
            m["bet"] = beta
        in_maps.append(m)

    global _saved_in_maps
    _saved_in_maps = in_maps
    res = run_bass_kernel_spmd(nc, in_maps, core_ids=list(range(8)))
    B = query.shape[0]
    full = np.empty((B, 2 * T, E), dtype=np.float32)
    for c in range(8):
        b, h = c // 2, c % 2
        full[b, h * T:(h + 1) * T] = res.results[c]["out"]
    return full


# revision 13
# speedup vs baseline: 1.3373x; 1.0813x over previous
"""Trainium2 Bass kernel for nn_MultiHeadAttention_5360119185803.

Full-d_model attention (no head split) + residual + LayerNorm, B=4, T=S=2048,
E=1024, fp32 in/out.

Sharding: 8 cores; core c owns batch b=c//2 and query rows
[(c%2)*1024, (c%2+1)*1024). K/V projections are split across the core pair
(each projects 1024 of the 2048 kv rows) and exchanged with a pair-wise
AllGather — halves the duplicated projection + transpose work.

Per-core device pipeline (all matmuls in float32r = TF32-like, full PE rate):
  P1  kT_half = (Wk.T).T @ xkh.T  [f, s_local]  -> DRAM, AllGather(pair)
      (activation transposes on PE via identity-matmul, fp32)
  P2  v_half  = xvh @ Wv.T        -> DRAM, AllGather(pair); bias bv folded
      into bo' = bo + Wo@bv on host (attn rows sum to 1)
  P3  qT = (Wq.T/32).T @ xq.T + bq/32  [f,t]  (1/sqrt(E) folded into Wq, bq)
  P4  load gathered kT; scoresT[s,t] = kT.T @ qT (PSUM) -> expT = exp(scoresT)
      (ACT; no max-subtraction: |scores/32| <~ 6 so exp is fp32-safe; bk
      dropped entirely — it shifts scores by a per-t constant, softmax-
      invariant). rowsum[1,t] = ones.T @ expT (PE), redistributed via DRAM.
  P5  ctxT[e',t] = sum_s v[s,e'] * expT[s,t]  (8 PSUM banks per t-half,
      v streamed from the gathered DRAM copy)
  P6  out[t,g] = (ctxT.T @ Wo.T) * (1/rowsum)[t] + bo' + residual; LayerNorm
      over g (bn_stats/bn_aggr on DVE, psum evict on ACT, bo-add on GpSimd);
      gamma/beta applied only if non-trivial.

kernel() is self-contained: host prep = shard + weight transposes/scale folds.
"""

import sys

sys.path.insert(0, "/opt/trn_rl_repo")

import numpy as np

import concourse.bacc as bacc
import concourse.bass as bass
import concourse.tile as tile
from concourse import mybir
from concourse.bass_utils import run_bass_kernel_spmd
from concourse.masks import make_identity

P = 128
E = 1024          # d_model
S = 2048          # kv seq len per batch
SH = S // 2       # kv rows projected locally
T = 1024          # query rows per core
NE = E // P       # 8 chunks of contraction dim
NT = T // P       # 8 t tiles
NS = S // P       # 16 s tiles
FD = 512          # matmul moving free dim / PSUM bank
NBLK_T = T // FD  # 2 blocks of 512

f32 = mybir.dt.float32
f32r = mybir.dt.float32r
AF = mybir.ActivationFunctionType
ALU = mybir.AluOpType
GROUPS = [[0, 1], [2, 3], [4, 5], [6, 7]]

_cache = {}


def _load_weight(nc, pool, dram):
    """[E, x] f32r DRAM -> [128, NE, x] f32r SBUF (HWDGE, split 2 queues)."""
    w = pool.tile([P, NE, E], f32r)
    v = dram.ap().rearrange("(j p) f -> j p f", p=P)
    for j in range(NE):
        eng = nc.sync if j % 2 == 0 else nc.scalar
        eng.dma_start(out=w[:, j, :], in_=v[j])
    return w


def _transpose_block(nc, xt_blk, x_dram, row0, nrows, nat_pool, tp_psum, ident):
    """xt_blk[:, j, :] (f32r [128, NE, nrows]) = x[row0:row0+nrows,
    j*128:(j+1)*128].T via PE identity-transpose + DVE psum evict."""
    for ss in range(nrows // P):
        nat = nat_pool.tile([P, E], f32)
        nc.sync.dma_start(out=nat, in_=x_dram.ap()[row0 + ss * P: row0 + (ss + 1) * P, :])
        for j in range(NE):
            ps = tp_psum.tile([P, P], f32)
            nc.tensor.transpose(ps, nat[:, j * P:(j + 1) * P], ident)
            nc.vector.tensor_copy(xt_blk[:, j, ss * P:(ss + 1) * P], ps)


def _build(apply_gb):
    nc = bacc.Bacc("TRN2", target_bir_lowering=False, debug=False, num_devices=8)

    xq = nc.dram_tensor("xq", [T, E], f32, kind="ExternalInput")
    xkh = nc.dram_tensor("xkh", [SH, E], f32, kind="ExternalInput")
    xvh = nc.dram_tensor("xvh", [SH, E], f32, kind="ExternalInput")
    wqt = nc.dram_tensor("wqt", [E, E], f32r, kind="ExternalInput")  # Wq.T/32 [e,f]
    wkt = nc.dram_tensor("wkt", [E, E], f32r, kind="ExternalInput")  # Wk.T   [e,f]
    wvt = nc.dram_tensor("wvt", [E, E], f32r, kind="ExternalInput")  # Wv.T   [e,e']
    wot = nc.dram_tensor("wot", [E, E], f32r, kind="ExternalInput")  # Wo.T   [e',g]
    bq2 = nc.dram_tensor("bq2", [P, NE], f32, kind="ExternalInput")  # bq/32 tiled
    bo2 = nc.dram_tensor("bo2", [E], f32, kind="ExternalInput")      # bo + Wo@bv
    if apply_gb:
        gam = nc.dram_tensor("gam", [E], f32, kind="ExternalInput")
        bet = nc.dram_tensor("bet", [E], f32, kind="ExternalInput")
    out = nc.dram_tensor("out", [T, E], f32, kind="ExternalOutput")

    kth = nc.dram_tensor("kth", [P, NE, SH], f32r)       # local kT half
    ktg = nc.dram_tensor("ktg", [2, P, NE, SH], f32r)    # gathered kT
    vh = nc.dram_tensor("vh", [SH, E], f32r)             # local v half
    vg = nc.dram_tensor("vg", [2, SH, E], f32r)          # gathered v
    rs_dram = nc.dram_tensor("rs_scratch", [T], f32)

    with tile.TileContext(nc) as tc:
        consts = tc.alloc_tile_pool(name="consts", bufs=1, side="left")
        eps_t = consts.tile([P, 1], f32)
        nc.vector.memset(eps_t, 1e-6)
        ones_f = consts.tile([P, 1], f32)
        nc.vector.memset(ones_f, 1.0)
        ones_r = consts.tile([P, 1], f32r)
        nc.vector.tensor_copy(ones_r, ones_f)
        recip_t = consts.tile([P, NT], f32)

        kT_pool = tc.alloc_tile_pool(name="kT", bufs=1, side="left")
        kT = kT_pool.tile([P, NE, S], f32r)  # [f, fchunk, s] 8MB
        qT_pool = tc.alloc_tile_pool(name="qT", bufs=1, side="left")
        qT = qT_pool.tile([P, NE, T], f32r)  # [f, fchunk, t] 4MB
        identp = tc.alloc_tile_pool(name="identp", bufs=1, side="left")
        ident = identp.tile([P, P], f32)
        make_identity(nc, ident)
        bq_sb = identp.tile([P, NE], f32)
        nc.sync.dma_start(out=bq_sb, in_=bq2.ap())

        # ---- P1: kT half projection -> DRAM -> pair AllGather ----
        with (
            tc.tile_pool(name="wk", bufs=1) as wkp,
            tc.tile_pool(name="p1nat", bufs=3) as natp,
            tc.tile_pool(name="p1xt", bufs=2) as xtp,
            tc.tile_pool(name="p1ev", bufs=3) as evp,
            tc.tile_pool(name="p1tp", bufs=4, space="PSUM") as tpp,
            tc.tile_pool(name="p1mm", bufs=4, space="PSUM") as mmp,
        ):
            wk_sb = _load_weight(nc, wkp, wkt)
            for sb in range(SH // FD):
                xt_blk = xtp.tile([P, NE, FD], f32r)
                _transpose_block(nc, xt_blk, xkh, sb * FD, FD, natp, tpp, ident)
                for ft in range(NE):
                    ps = mmp.tile([P, FD], f32)
                    for j in range(NE):
                        nc.tensor.matmul(ps, wk_sb[:, j, ft * P:(ft + 1) * P],
                                         xt_blk[:, j, :],
                                         start=(j == 0), stop=(j == NE - 1))
                    ev = evp.tile([P, FD], f32r)
                    nc.vector.tensor_copy(ev, ps)
                    nc.scalar.dma_start(
                        out=kth.ap()[:, ft, sb * FD:(sb + 1) * FD], in_=ev)
            nc.gpsimd.collective_compute(
                "AllGather", ALU.bypass, replica_groups=GROUPS,
                ins=[kth.ap()], outs=[ktg.ap()])
            # prefetch gathered kT into SBUF during P2/P3 (consumed in P4)
            for r in range(2):
                for j in range(NE):
                    eng = nc.sync if j % 2 == 0 else nc.scalar
                    eng.dma_start(out=kT[:, j, r * SH:(r + 1) * SH],
                                  in_=ktg.ap()[r, :, j, :])

        # ---- P2: v half projection -> DRAM -> pair AllGather ----
        with (
            tc.tile_pool(name="wv", bufs=1) as wvp,
            tc.tile_pool(name="p2nat", bufs=3) as natp,
            tc.tile_pool(name="p2xt", bufs=2) as xtp,
            tc.tile_pool(name="p2ev", bufs=3) as evp,
            tc.tile_pool(name="p2tp", bufs=4, space="PSUM") as tpp,
            tc.tile_pool(name="p2mm", bufs=4, space="PSUM") as mmp,
        ):
            wv_sb = _load_weight(nc, wvp, wvt)
            for sb in range(SH // FD):
                xt_blk = xtp.tile([P, NE, FD], f32r)
                _transpose_block(nc, xt_blk, xvh, sb * FD, FD, natp, tpp, ident)
                for ss in range(FD // P):
                    ev = evp.tile([P, E], f32r)
                    for ec in range(E // FD):
                        ps = mmp.tile([P, FD], f32)
                        for j in range(NE):
                            nc.tensor.matmul(ps, xt_blk[:, j, ss * P:(ss + 1) * P],
                                             wv_sb[:, j, ec * FD:(ec + 1) * FD],
                                             start=(j == 0), stop=(j == NE - 1))
                        nc.vector.tensor_copy(ev[:, ec * FD:(ec + 1) * FD], ps)
                    r0 = sb * FD + ss * P
                    nc.scalar.dma_start(out=vh.ap()[r0:r0 + P, :], in_=ev)
            nc.gpsimd.collective_compute(
                "AllGather", ALU.bypass, replica_groups=GROUPS,
                ins=[vh.ap()], outs=[vg.ap()])

        # ---- P3: qT projection (+bq/32) ----
        with (
            tc.tile_pool(name="wq", bufs=1) as wqp,
            tc.tile_pool(name="p3nat", bufs=3) as natp,
            tc.tile_pool(name="p3xt", bufs=2) as xtp,
            tc.tile_pool(name="p3tp", bufs=4, space="PSUM") as tpp,
            tc.tile_pool(name="p3mm", bufs=4, space="PSUM") as mmp,
        ):
            wq_sb = _load_weight(nc, wqp, wqt)
            for tb in range(NBLK_T):
                xt_blk = xtp.tile([P, NE, FD], f32r)
                _transpose_block(nc, xt_blk, xq, tb * FD, FD, natp, tpp, ident)
                for ft in range(NE):
                    ps = mmp.tile([P, FD], f32)
                    for j in range(NE):
                        nc.tensor.matmul(ps, wq_sb[:, j, ft * P:(ft + 1) * P],
                                         xt_blk[:, j, :],
                                         start=(j == 0), stop=(j == NE - 1))
                    nc.vector.tensor_scalar(
                        out=qT[:, ft, tb * FD:(tb + 1) * FD], in0=ps,
                        scalar1=bq_sb[:, ft:ft + 1], scalar2=None, op0=ALU.add)
        identp.release()

        # ---- P4: scoresT -> expT; rowsum -> recip ----
        ctxT_pool = tc.alloc_tile_pool(name="ctxT", bufs=1, side="right")
        ctxT = ctxT_pool.tile([P, NE, T], f32r)  # [e', echunk, t] 4MB
        expT_pool = tc.alloc_tile_pool(name="expT", bufs=1, side="right")
        expT = expT_pool.tile([P, NS, T], f32r)  # [s, stile, t] 8MB
        with tc.tile_pool(name="p4mm", bufs=4, space="PSUM") as mmp:
            for st in range(NS):
                for tb in range(NBLK_T):
                    ps = mmp.tile([P, FD], f32)
                    for j in range(NE):
                        nc.tensor.matmul(ps, kT[:, j, st * P:(st + 1) * P],
                                         qT[:, j, tb * FD:(tb + 1) * FD],
                                         start=(j == 0), stop=(j == NE - 1))
                    nc.scalar.activation(expT[:, st, tb * FD:(tb + 1) * FD], ps, AF.Exp)

        with (
            tc.tile_pool(name="p4rs", bufs=2, space="PSUM") as rsp,
            tc.tile_pool(name="p4rw", bufs=1, side="right") as rwp,
        ):
            rs_sb = rwp.tile([1, T], f32)
            for tb in range(NBLK_T):
                ps = rsp.tile([P, FD], f32)
                for st in range(NS):
                    nc.tensor.matmul(ps[0:1, :], ones_r[:, 0:1],
                                     expT[:, st, tb * FD:(tb + 1) * FD],
                                     start=(st == 0), stop=(st == NS - 1))
                nc.vector.tensor_copy(rs_sb[0:1, tb * FD:(tb + 1) * FD], ps[0:1, :])
            nc.scalar.dma_start(out=rs_dram.ap(), in_=rs_sb[0:1, :])
            rsT = rwp.tile([P, NT], f32)
            nc.scalar.dma_start(out=rsT, in_=rs_dram.ap().rearrange("(j p) -> p j", p=P))
            nc.vector.reciprocal(recip_t, rsT)

        qT_pool.release()
        kT_pool.release()

        # ---- P5: ctxT ----
        vflat = vg.ap().rearrange("r s e -> (r s) e")
        with (
            tc.tile_pool(name="p5v", bufs=3, side="right") as vp,
            tc.tile_pool(name="p5mm", bufs=1, space="PSUM") as mmp,
        ):
            for tb in range(NBLK_T):
                pss = [mmp.tile([P, FD], f32, name=f"ctxps{tb}_{e}",
                                tag=f"ctxps{e}") for e in range(NE)]
                for st in range(NS):
                    vt = vp.tile([P, E], f32r)
                    nc.scalar.dma_start(out=vt, in_=vflat[st * P:(st + 1) * P, :])
                    for e in range(NE):
                        nc.tensor.matmul(pss[e], vt[:, e * P:(e + 1) * P],
                                         expT[:, st, tb * FD:(tb + 1) * FD],
                                         start=(st == 0), stop=(st == NS - 1))
                for e in range(NE):
                    nc.vector.tensor_copy(ctxT[:, e, tb * FD:(tb + 1) * FD], pss[e])
        expT_pool.release()

        # ---- P6: out projection + residual + LayerNorm ----
        with (
            tc.tile_pool(name="wo", bufs=1, side="right") as wop,
            tc.tile_pool(name="p6c", bufs=1, side="right") as p6c,
            tc.tile_pool(name="p6res", bufs=2, side="right") as resp,
            tc.tile_pool(name="p6y", bufs=2, side="right") as yp,
            tc.tile_pool(name="p6ln", bufs=4, side="right") as lnp,
            tc.tile_pool(name="p6out", bufs=2, side="right") as outp,
            tc.tile_pool(name="p6mm", bufs=3, space="PSUM") as mmp,
        ):
            wo_sb = _load_weight(nc, wop, wot)
            bo_sb = p6c.tile([P, E], f32)
            nc.gpsimd.dma_start(out=bo_sb, in_=bo2.ap().partition_broadcast(P))
            if apply_gb:
                gam_sb = p6c.tile([P, E], f32)
                nc.gpsimd.dma_start(out=gam_sb, in_=gam.ap().partition_broadcast(P))
                bet_sb = p6c.tile([P, E], f32)
                nc.gpsimd.dma_start(out=bet_sb, in_=bet.ap().partition_broadcast(P))
            for tt in range(NT):
                y = yp.tile([P, E], f32)
                for gc in range(E // FD):
                    ps = mmp.tile([P, FD], f32)
                    for j in range(NE):
                        nc.tensor.matmul(ps, ctxT[:, j, tt * P:(tt + 1) * P],
                                         wo_sb[:, j, gc * FD:(gc + 1) * FD],
                                         start=(j == 0), stop=(j == NE - 1))
                    # y = psum / rowsum  (ACT: Copy(in*scale))
                    nc.scalar.activation(y[:, gc * FD:(gc + 1) * FD], ps, AF.Copy,
                                         scale=recip_t[:, tt:tt + 1])
                res = resp.tile([P, E], f32)
                nc.sync.dma_start(out=res, in_=xq.ap()[tt * P:(tt + 1) * P, :])
                nc.gpsimd.tensor_tensor(y, y, bo_sb, ALU.add)
                nc.vector.tensor_add(y, y, res)
                stats = lnp.tile([P, 2, 6], f32)
                nc.vector.bn_stats(stats[:, 0, :], y[:, 0:FD])
                nc.vector.bn_stats(stats[:, 1, :], y[:, FD:E])
                mv = lnp.tile([P, 2], f32)
                nc.vector.bn_aggr(mv, stats)
                rstd = lnp.tile([P, 1], f32)
                nc.scalar.activation(rstd, mv[:, 1:2], AF.Sqrt, bias=eps_t)
                nc.vector.reciprocal(rstd, rstd)
                o = outp.tile([P, E], f32)
                nc.vector.tensor_scalar(out=o, in0=y, scalar1=mv[:, 0:1],
                                        scalar2=rstd, op0=ALU.subtract, op1=ALU.mult)
                if apply_gb:
                    nc.vector.tensor_mul(o, o, gam_sb)
                    nc.vector.tensor_add(o, o, bet_sb)
                nc.sync.dma_start(out=out.ap()[tt * P:(tt + 1) * P, :], in_=o)

        ctxT_pool.release()
        consts.release()

    nc.compile()
    return nc


def kernel(query, key, value, Wq, bq, Wk, bk, Wv, bv, Wo, bo, gamma, beta):
    query = np.asarray(query, dtype=np.float32)
    key = np.asarray(key, dtype=np.float32)
    value = np.asarray(value, dtype=np.float32)
    Wq = np.asarray(Wq, dtype=np.float32)
    bq = np.asarray(bq, dtype=np.float32)
    Wv = np.asarray(Wv, dtype=np.float32)
    bv = np.asarray(bv, dtype=np.float32)
    Wk = np.asarray(Wk, dtype=np.float32)
    Wo = np.asarray(Wo, dtype=np.float32)
    bo = np.asarray(bo, dtype=np.float32)
    gamma = np.asarray(gamma, dtype=np.float32)
    beta = np.asarray(beta, dtype=np.float32)

    scale = np.float32(1.0) / np.float32(np.sqrt(np.float32(E)))
    wqt = np.ascontiguousarray(Wq.T) * scale
    wkt = np.ascontiguousarray(Wk.T)
    wvt = np.ascontiguousarray(Wv.T)
    wot = np.ascontiguousarray(Wo.T)
    bq2 = np.ascontiguousarray((bq * scale).reshape(NE, P).T)
    bo2 = (bo + Wo @ bv).astype(np.float32)
    apply_gb = not (np.all(gamma == 1.0) and np.all(beta == 0.0))

    if apply_gb not in _cache:
        _cache[apply_gb] = _build(apply_gb)
    nc = _cache[apply_gb]

    in_maps = []
    for c in range(8):
        b, h = c // 2, c % 2
        m = {
            "xq": np.ascontiguousarray(query[b, h * T:(h + 1) * T]),
            "xkh": np.ascontiguousarray(key[b, h * SH:(h + 1) * SH]),
            "xvh": np.ascontiguousarray(value[b, h * SH:(h + 1) * SH]),
            "wqt": wqt, "wkt": wkt, "wvt": wvt, "wot": wot,
            "bq2": bq2, "bo2": bo2,
        }
        if apply_gb:
            m["gam"] = gamma
            m["bet"] = beta
        in_maps.append(m)

    global _saved_in_maps
    _saved_in_maps = in_maps
    res = run_bass_kernel_spmd(nc, in_maps, core_ids=list(range(8)))
    B = query.shape[0]
    full = np.empty((B, 2 * T, E), dtype=np.float32)
    for c in range(8):
        b, h = c // 2, c % 2
        full[b, h * T:(h + 1) * T] = res.results[c]["out"]
    return full


# revision 14
# speedup vs baseline: 1.3428x; 1.0041x over previous
"""Trainium2 Bass kernel for nn_MultiHeadAttention_5360119185803.

Full-d_model attention (no head split) + residual + LayerNorm, B=4, T=S=2048,
E=1024, fp32 in/out.

Sharding: 8 cores; core c owns batch b=c//2 and query rows
[(c%2)*1024, (c%2+1)*1024). K/V projections are split across the core pair
(each projects 1024 of the 2048 kv rows) and exchanged with a pair-wise
AllGather — halves the duplicated projection + transpose work.

Per-core device pipeline (all matmuls in float32r = TF32-like, full PE rate):
  P1  kT_half = (Wk.T).T @ xkh.T  [f, s_local]  -> DRAM, AllGather(pair)
      (activation transposes on PE via identity-matmul, fp32)
  P2  v_half  = xvh @ Wv.T        -> DRAM, AllGather(pair); bias bv folded
      into bo' = bo + Wo@bv on host (attn rows sum to 1)
  P3  qT = (Wq.T/32).T @ xq.T + bq/32  [f,t]  (1/sqrt(E) folded into Wq, bq)
  P4  load gathered kT; scoresT[s,t] = kT.T @ qT (PSUM) -> expT = exp(scoresT)
      (ACT; no max-subtraction: |scores/32| <~ 6 so exp is fp32-safe; bk
      dropped entirely — it shifts scores by a per-t constant, softmax-
      invariant). rowsum[1,t] = ones.T @ expT (PE), redistributed via DRAM.
  P5  ctxT[e',t] = sum_s v[s,e'] * expT[s,t]  (8 PSUM banks per t-half,
      v streamed from the gathered DRAM copy)
  P6  out[t,g] = (ctxT.T @ Wo.T) * (1/rowsum)[t] + bo' + residual; LayerNorm
      over g (bn_stats/bn_aggr on DVE, psum evict on ACT, bo-add on GpSimd);
      gamma/beta applied only if non-trivial.

kernel() is self-contained: host prep = shard + weight transposes/scale folds.
"""

import sys

sys.path.insert(0, "/opt/trn_rl_repo")

import numpy as np

import concourse.bacc as bacc
import concourse.bass as bass
import concourse.tile as tile
from concourse import mybir
from concourse.bass_utils import run_bass_kernel_spmd
from concourse.masks import make_identity

P = 128
E = 1024          # d_model
S = 2048          # kv seq len per batch
SH = S // 2       # kv rows projected locally
T = 1024          # query rows per core
NE = E // P       # 8 chunks of contraction dim
NT = T // P       # 8 t tiles
NS = S // P       # 16 s tiles
FD = 512          # matmul moving free dim / PSUM bank
NBLK_T = T // FD  # 2 blocks of 512

f32 = mybir.dt.float32
f32r = mybir.dt.float32r
AF = mybir.ActivationFunctionType
ALU = mybir.AluOpType
GROUPS = [[0, 1], [2, 3], [4, 5], [6, 7]]

_cache = {}


def _load_weight(nc, pool, dram):
    """[E, x] f32r DRAM -> [128, NE, x] f32r SBUF (HWDGE, split 2 queues)."""
    w = pool.tile([P, NE, E], f32r)
    v = dram.ap().rearrange("(j p) f -> j p f", p=P)
    for j in range(NE):
        eng = nc.sync if j % 2 == 0 else nc.scalar
        eng.dma_start(out=w[:, j, :], in_=v[j])
    return w


def _transpose_block(nc, xt_blk, x_dram, row0, nrows, nat_pool, tp_psum, ident):
    """xt_blk[:, j, :] (f32r [128, NE, nrows]) = x[row0:row0+nrows,
    j*128:(j+1)*128].T via PE identity-transpose + DVE psum evict."""
    for ss in range(nrows // P):
        nat = nat_pool.tile([P, E], f32)
        nc.sync.dma_start(out=nat, in_=x_dram.ap()[row0 + ss * P: row0 + (ss + 1) * P, :])
        for j in range(NE):
            ps = tp_psum.tile([P, P], f32)
            nc.tensor.transpose(ps, nat[:, j * P:(j + 1) * P], ident)
            nc.vector.tensor_copy(xt_blk[:, j, ss * P:(ss + 1) * P], ps)


def _build(apply_gb):
    nc = bacc.Bacc("TRN2", target_bir_lowering=False, debug=False, num_devices=8)

    xq = nc.dram_tensor("xq", [T, E], f32, kind="ExternalInput")
    xkh = nc.dram_tensor("xkh", [SH, E], f32, kind="ExternalInput")
    xvh = nc.dram_tensor("xvh", [SH, E], f32, kind="ExternalInput")
    wqt = nc.dram_tensor("wqt", [E, E], f32r, kind="ExternalInput")  # Wq.T/32 [e,f]
    wkt = nc.dram_tensor("wkt", [E, E], f32r, kind="ExternalInput")  # Wk.T   [e,f]
    wvt = nc.dram_tensor("wvt", [E, E], f32r, kind="ExternalInput")  # Wv.T   [e,e']
    wot = nc.dram_tensor("wot", [E, E], f32r, kind="ExternalInput")  # Wo.T   [e',g]
    bq2 = nc.dram_tensor("bq2", [P, NE], f32, kind="ExternalInput")  # bq/32 tiled
    bo2 = nc.dram_tensor("bo2", [E], f32, kind="ExternalInput")      # bo + Wo@bv
    if apply_gb:
        gam = nc.dram_tensor("gam", [E], f32, kind="ExternalInput")
        bet = nc.dram_tensor("bet", [E], f32, kind="ExternalInput")
    out = nc.dram_tensor("out", [T, E], f32, kind="ExternalOutput")

    kth = nc.dram_tensor("kth", [P, NE, SH], f32r)       # local kT half
    ktg = nc.dram_tensor("ktg", [2, P, NE, SH], f32r)    # gathered kT
    vh = nc.dram_tensor("vh", [SH, E], f32r)             # local v half
    vg = nc.dram_tensor("vg", [2, SH, E], f32r)          # gathered v
    rs_dram = nc.dram_tensor("rs_scratch", [T], f32)

    with tile.TileContext(nc) as tc:
        consts = tc.alloc_tile_pool(name="consts", bufs=1, side="left")
        eps_t = consts.tile([P, 1], f32)
        nc.vector.memset(eps_t, 1e-6)
        ones_f = consts.tile([P, 1], f32)
        nc.vector.memset(ones_f, 1.0)
        ones_r = consts.tile([P, 1], f32r)
        nc.vector.tensor_copy(ones_r, ones_f)
        recip_t = consts.tile([P, NT], f32)

        kT_pool = tc.alloc_tile_pool(name="kT", bufs=1, side="left")
        kT = kT_pool.tile([P, NE, S], f32r)  # [f, fchunk, s] 8MB
        qT_pool = tc.alloc_tile_pool(name="qT", bufs=1, side="left")
        qT = qT_pool.tile([P, NE, T], f32r)  # [f, fchunk, t] 4MB
        identp = tc.alloc_tile_pool(name="identp", bufs=1, side="left")
        ident = identp.tile([P, P], f32)
        make_identity(nc, ident)
        bq_sb = identp.tile([P, NE], f32)
        nc.sync.dma_start(out=bq_sb, in_=bq2.ap())

        # ---- P1: kT half projection -> DRAM -> pair AllGather ----
        with (
            tc.tile_pool(name="wk", bufs=1) as wkp,
            tc.tile_pool(name="p1nat", bufs=3) as natp,
            tc.tile_pool(name="p1xt", bufs=2) as xtp,
            tc.tile_pool(name="p1ev", bufs=3) as evp,
            tc.tile_pool(name="p1tp", bufs=4, space="PSUM") as tpp,
            tc.tile_pool(name="p1mm", bufs=4, space="PSUM") as mmp,
        ):
            wk_sb = _load_weight(nc, wkp, wkt)
            for sb in range(SH // FD):
                xt_blk = xtp.tile([P, NE, FD], f32r)
                _transpose_block(nc, xt_blk, xkh, sb * FD, FD, natp, tpp, ident)
                for ft in range(NE):
                    ps = mmp.tile([P, FD], f32)
                    for j in range(NE):
                        nc.tensor.matmul(ps, wk_sb[:, j, ft * P:(ft + 1) * P],
                                         xt_blk[:, j, :],
                                         start=(j == 0), stop=(j == NE - 1))
                    ev = evp.tile([P, FD], f32r)
                    nc.vector.tensor_copy(ev, ps)
                    nc.scalar.dma_start(
                        out=kth.ap()[:, ft, sb * FD:(sb + 1) * FD], in_=ev)
            nc.gpsimd.collective_compute(
                "AllGather", ALU.bypass, replica_groups=GROUPS,
                ins=[kth.ap()], outs=[ktg.ap()])

        # ---- P2: v half projection -> DRAM -> pair AllGather ----
        with (
            tc.tile_pool(name="wv", bufs=1) as wvp,
            tc.tile_pool(name="p2nat", bufs=3) as natp,
            tc.tile_pool(name="p2xt", bufs=2) as xtp,
            tc.tile_pool(name="p2ev", bufs=3) as evp,
            tc.tile_pool(name="p2tp", bufs=4, space="PSUM") as tpp,
            tc.tile_pool(name="p2mm", bufs=4, space="PSUM") as mmp,
        ):
            wv_sb = _load_weight(nc, wvp, wvt)
            for sb in range(SH // FD):
                xt_blk = xtp.tile([P, NE, FD], f32r)
                _transpose_block(nc, xt_blk, xvh, sb * FD, FD, natp, tpp, ident)
                for ss in range(FD // P):
                    ev = evp.tile([P, E], f32r)
                    for ec in range(E // FD):
                        ps = mmp.tile([P, FD], f32)
                        for j in range(NE):
                            nc.tensor.matmul(ps, xt_blk[:, j, ss * P:(ss + 1) * P],
                                             wv_sb[:, j, ec * FD:(ec + 1) * FD],
                                             start=(j == 0), stop=(j == NE - 1))
                        nc.vector.tensor_copy(ev[:, ec * FD:(ec + 1) * FD], ps)
                    r0 = sb * FD + ss * P
                    nc.scalar.dma_start(out=vh.ap()[r0:r0 + P, :], in_=ev)
            nc.gpsimd.collective_compute(
                "AllGather", ALU.bypass, replica_groups=GROUPS,
                ins=[vh.ap()], outs=[vg.ap()])
            # prefetch gathered kT into SBUF during P3 (consumed in P4).
            # On the gpsimd queue: waits only on AG(k), which is long done,
            # and keeps the in-order HWDGE queues free for P3's loads.
            for r in range(2):
                for j in range(NE):
                    nc.gpsimd.dma_start(out=kT[:, j, r * SH:(r + 1) * SH],
                                        in_=ktg.ap()[r, :, j, :])

        # ---- P3: qT projection (+bq/32) ----
        with (
            tc.tile_pool(name="wq", bufs=1) as wqp,
            tc.tile_pool(name="p3nat", bufs=3) as natp,
            tc.tile_pool(name="p3xt", bufs=2) as xtp,
            tc.tile_pool(name="p3tp", bufs=4, space="PSUM") as tpp,
            tc.tile_pool(name="p3mm", bufs=4, space="PSUM") as mmp,
        ):
            wq_sb = _load_weight(nc, wqp, wqt)
            for tb in range(NBLK_T):
                xt_blk = xtp.tile([P, NE, FD], f32r)
                _transpose_block(nc, xt_blk, xq, tb * FD, FD, natp, tpp, ident)
                for ft in range(NE):
                    ps = mmp.tile([P, FD], f32)
                    for j in range(NE):
                        nc.tensor.matmul(ps, wq_sb[:, j, ft * P:(ft + 1) * P],
                                         xt_blk[:, j, :],
                                         start=(j == 0), stop=(j == NE - 1))
                    nc.vector.tensor_scalar(
                        out=qT[:, ft, tb * FD:(tb + 1) * FD], in0=ps,
                        scalar1=bq_sb[:, ft:ft + 1], scalar2=None, op0=ALU.add)
        identp.release()

        # ---- P4: scoresT -> expT; rowsum -> recip ----
        ctxT_pool = tc.alloc_tile_pool(name="ctxT", bufs=1, side="right")
        ctxT = ctxT_pool.tile([P, NE, T], f32r)  # [e', echunk, t] 4MB
        expT_pool = tc.alloc_tile_pool(name="expT", bufs=1, side="right")
        expT = expT_pool.tile([P, NS, T], f32r)  # [s, stile, t] 8MB
        with tc.tile_pool(name="p4mm", bufs=4, space="PSUM") as mmp:
            for st in range(NS):
                for tb in range(NBLK_T):
                    ps = mmp.tile([P, FD], f32)
                    for j in range(NE):
                        nc.tensor.matmul(ps, kT[:, j, st * P:(st + 1) * P],
                                         qT[:, j, tb * FD:(tb + 1) * FD],
                                         start=(j == 0), stop=(j == NE - 1))
                    nc.scalar.activation(expT[:, st, tb * FD:(tb + 1) * FD], ps, AF.Exp)

        with (
            tc.tile_pool(name="p4rs", bufs=2, space="PSUM") as rsp,
            tc.tile_pool(name="p4rw", bufs=1, side="right") as rwp,
        ):
            rs_sb = rwp.tile([1, T], f32)
            for tb in range(NBLK_T):
                ps = rsp.tile([P, FD], f32)
                for st in range(NS):
                    nc.tensor.matmul(ps[0:1, :], ones_r[:, 0:1],
                                     expT[:, st, tb * FD:(tb + 1) * FD],
                                     start=(st == 0), stop=(st == NS - 1))
                nc.vector.tensor_copy(rs_sb[0:1, tb * FD:(tb + 1) * FD], ps[0:1, :])
            nc.scalar.dma_start(out=rs_dram.ap(), in_=rs_sb[0:1, :])
            rsT = rwp.tile([P, NT], f32)
            nc.scalar.dma_start(out=rsT, in_=rs_dram.ap().rearrange("(j p) -> p j", p=P))
            nc.vector.reciprocal(recip_t, rsT)

        qT_pool.release()
        kT_pool.release()

        # ---- P5: ctxT ----
        vflat = vg.ap().rearrange("r s e -> (r s) e")
        with (
            tc.tile_pool(name="p5v", bufs=3, side="right") as vp,
            tc.tile_pool(name="p5mm", bufs=1, space="PSUM") as mmp,
        ):
            for tb in range(NBLK_T):
                pss = [mmp.tile([P, FD], f32, name=f"ctxps{tb}_{e}",
                                tag=f"ctxps{e}") for e in range(NE)]
                for st in range(NS):
                    vt = vp.tile([P, E], f32r)
                    nc.scalar.dma_start(out=vt, in_=vflat[st * P:(st + 1) * P, :])
                    for e in range(NE):
                        nc.tensor.matmul(pss[e], vt[:, e * P:(e + 1) * P],
                                         expT[:, st, tb * FD:(tb + 1) * FD],
                                         start=(st == 0), stop=(st == NS - 1))
                for e in range(NE):
                    nc.vector.tensor_copy(ctxT[:, e, tb * FD:(tb + 1) * FD], pss[e])
        expT_pool.release()

        # ---- P6: out projection + residual + LayerNorm ----
        with (
            tc.tile_pool(name="wo", bufs=1, side="right") as wop,
            tc.tile_pool(name="p6c", bufs=1, side="right") as p6c,
            tc.tile_pool(name="p6res", bufs=2, side="right") as resp,
            tc.tile_pool(name="p6y", bufs=2, side="right") as yp,
            tc.tile_pool(name="p6ln", bufs=4, side="right") as lnp,
            tc.tile_pool(name="p6out", bufs=2, side="right") as outp,
            tc.tile_pool(name="p6mm", bufs=3, space="PSUM") as mmp,
        ):
            wo_sb = _load_weight(nc, wop, wot)
            bo_sb = p6c.tile([P, E], f32)
            nc.gpsimd.dma_start(out=bo_sb, in_=bo2.ap().partition_broadcast(P))
            if apply_gb:
                gam_sb = p6c.tile([P, E], f32)
                nc.gpsimd.dma_start(out=gam_sb, in_=gam.ap().partition_broadcast(P))
                bet_sb = p6c.tile([P, E], f32)
                nc.gpsimd.dma_start(out=bet_sb, in_=bet.ap().partition_broadcast(P))
            for tt in range(NT):
                y = yp.tile([P, E], f32)
                for gc in range(E // FD):
                    ps = mmp.tile([P, FD], f32)
                    for j in range(NE):
                        nc.tensor.matmul(ps, ctxT[:, j, tt * P:(tt + 1) * P],
                                         wo_sb[:, j, gc * FD:(gc + 1) * FD],
                                         start=(j == 0), stop=(j == NE - 1))
                    # y = psum / rowsum  (ACT: Copy(in*scale))
                    nc.scalar.activation(y[:, gc * FD:(gc + 1) * FD], ps, AF.Copy,
                                         scale=recip_t[:, tt:tt + 1])
                res = resp.tile([P, E], f32)
                nc.sync.dma_start(out=res, in_=xq.ap()[tt * P:(tt + 1) * P, :])
                nc.gpsimd.tensor_tensor(y, y, bo_sb, ALU.add)
                nc.vector.tensor_add(y, y, res)
                stats = lnp.tile([P, 2, 6], f32)
                nc.vector.bn_stats(stats[:, 0, :], y[:, 0:FD])
                nc.vector.bn_stats(stats[:, 1, :], y[:, FD:E])
                mv = lnp.tile([P, 2], f32)
                nc.vector.bn_aggr(mv, stats)
                rstd = lnp.tile([P, 1], f32)
                nc.scalar.activation(rstd, mv[:, 1:2], AF.Sqrt, bias=eps_t)
                nc.vector.reciprocal(rstd, rstd)
                o = outp.tile([P, E], f32)
                nc.vector.tensor_scalar(out=o, in0=y, scalar1=mv[:, 0:1],
                                        scalar2=rstd, op0=ALU.subtract, op1=ALU.mult)
                if apply_gb:
                    nc.vector.tensor_mul(o, o, gam_sb)
                    nc.vector.tensor_add(o, o, bet_sb)
                nc.sync.dma_start(out=out.ap()[tt * P:(tt + 1) * P, :], in_=o)

        ctxT_pool.release()
        consts.release()

    nc.compile()
    return nc


def kernel(query, key, value, Wq, bq, Wk, bk, Wv, bv, Wo, bo, gamma, beta):
    query = np.asarray(query, dtype=np.float32)
    key = np.asarray(key, dtype=np.float32)
    value = np.asarray(value, dtype=np.float32)
    Wq = np.asarray(Wq, dtype=np.float32)
    bq = np.asarray(bq, dtype=np.float32)
    Wv = np.asarray(Wv, dtype=np.float32)
    bv = np.asarray(bv, dtype=np.float32)
    Wk = np.asarray(Wk, dtype=np.float32)
    Wo = np.asarray(Wo, dtype=np.float32)
    bo = np.asarray(bo, dtype=np.float32)
    gamma = np.asarray(gamma, dtype=np.float32)
    beta = np.asarray(beta, dtype=np.float32)

    scale = np.float32(1.0) / np.float32(np.sqrt(np.float32(E)))
    wqt = np.ascontiguousarray(Wq.T) * scale
    wkt = np.ascontiguousarray(Wk.T)
    wvt = np.ascontiguousarray(Wv.T)
    wot = np.ascontiguousarray(Wo.T)
    bq2 = np.ascontiguousarray((bq * scale).reshape(NE, P).T)
    bo2 = (bo + Wo @ bv).astype(np.float32)
    apply_gb = not (np.all(gamma == 1.0) and np.all(beta == 0.0))

    if apply_gb not in _cache:
        _cache[apply_gb] = _build(apply_gb)
    nc = _cache[apply_gb]

    in_maps = []
    for c in range(8):
        b, h = c // 2, c % 2
        m = {
            "xq": np.ascontiguousarray(query[b, h * T:(h + 1) * T]),
            "xkh": np.ascontiguousarray(key[b, h * SH:(h + 1) * SH]),
            "xvh": np.ascontiguousarray(value[b, h * SH:(h + 1) * SH]),
            "wqt": wqt, "wkt": wkt, "wvt": wvt, "wot": wot,
            "bq2": bq2, "bo2": bo2,
        }
        if apply_gb:
            m["gam"] = gamma
            m["bet"] = beta
        in_maps.append(m)

    global _saved_in_maps
    _saved_in_maps = in_maps
    res = run_bass_kernel_spmd(nc, in_maps, core_ids=list(range(8)))
    B = query.shape[0]
    full = np.empty((B, 2 * T, E), dtype=np.float32)
    for c in range(8):
        b, h = c // 2, c % 2
        full[b, h * T:(h + 1) * T] = res.results[c]["out"]
    return full
